# revision 18
# baseline (speedup 1.0000x reference)
"""ForgetMult linear recurrence h_t = f_t*x_t + (1-f_t)*h_{t-1} on 8 trn2 cores.

Sharding: batch dim B=64 split across 8 cores (8 batches/core). Per core the
C = 8*1024 = 8192 (b,h) channels are independent scans over T=1024.

All wire traffic fp16 (host pre-transposes to channel-major [C, T]).
Everything runs on DVE + ACT only: concurrent GpSimd SBUF traffic slows DVE
scans ~2.3x (SBUF port contention), and at the quiet rate the scan costs
~187ns + 2.02 cyc/elem.

Groups are processed in merged scans: several groups share one long scan
instruction, separated by reset columns with a=0, b=h0_next — the scan
state = a*state + b makes that an EXACT re-initialization. This amortizes
per-instruction overhead for both the scan and the 2x-mode b=f*x multiply.
"""

import numpy as np

import concourse.bacc as bacc
import concourse.bass as bass
import concourse.mybir as mybir
from concourse import bass_utils
from concourse.tile import TileContext

T = 1024
B = 64
H = 1024
NCORES = 8
BS = B // NCORES  # batches per core
C = BS * H  # channels per core (independent scans)
G = 128  # channels per group == partition dim
NG = C // G  # channel groups per core
STRIDE = T + 1  # group data + one reset column

F32 = mybir.dt.float32
F16 = mybir.dt.float16

# Merge plan: small first merge so the pipeline ramps quickly, 4-group
# merges in steady state, singles at the end (the last one split so its
# output DMA overlaps its second-half scan).
MERGES = [[0, 1]] + [list(range(2 + 4 * k, 6 + 4 * k)) for k in range(15)]
MERGES += [[62], [63]]


def build_program() -> bass.Bass:
    nc = bacc.Bacc(trn_type="TRN2")
    f_d = nc.dram_tensor("f", (C, T), F16, kind="ExternalInput")
    x_d = nc.dram_tensor("x", (C, T), F16, kind="ExternalInput")
    h0_d = nc.dram_tensor("h0", (G, NG), F32, kind="ExternalInput")
    y_d = nc.dram_tensor("y", (C, T), F16, kind="ExternalOutput")

    COPY = mybir.ActivationFunctionType.Copy
    MULT = mybir.AluOpType.mult
    ADD = mybir.AluOpType.add
    HALF = T // 2

    with TileContext(nc) as tc:
        with (
            tc.tile_pool(name="consts", bufs=1) as consts,
            tc.tile_pool(name="io", bufs=4) as io,
            tc.tile_pool(name="mid", bufs=3) as mid,
            tc.tile_pool(name="hpool", bufs=3) as hpool,
        ):
            h0t = consts.tile([G, NG], F32)
            zcol = consts.tile([G, 1], F16)
            nc.gpsimd.memset(zcol[:, :], 0.0)

            first = True
            for mg in MERGES:
                n = len(mg)
                W = n * STRIDE - 1  # last group needs no reset column
                g0 = mg[0]
                ft = io.tile([G, W], F16, tag="f", name=f"f{g0}")
                xt = io.tile([G, W], F16, tag="x", name=f"x{g0}")
                at = mid.tile([G, W], F16, tag="a", name=f"a{g0}")
                bt = mid.tile([G, W], F16, tag="b", name=f"b{g0}")
                ht = hpool.tile([G, W], F16, tag="h", name=f"h{g0}")
                for i, g in enumerate(mg):
                    rows = slice(g * G, (g + 1) * G)
                    cl = slice(i * STRIDE, i * STRIDE + T)
                    nc.sync.dma_start(out=ft[:, cl], in_=f_d[rows, :])
                    nc.sync.dma_start(out=xt[:, cl], in_=x_d[rows, :])
                    if first:
                        # h0 load rides behind the first data tiles
                        nc.sync.dma_start(out=h0t[:, :], in_=h0_d[:, :])
                        first = False
                    # a = 1 - f on the data slice only (reset cols untouched)
                    nc.scalar.activation(
                        at[:, cl], ft[:, cl], COPY, bias=1.0, scale=-1.0
                    )
                # b = f*x full width on DVE 2x mode (junk in reset cols)
                nc.vector.tensor_tensor(
                    out=bt[:, :], in0=ft[:, :], in1=xt[:, :], op=MULT
                )
                # reset columns: a=0, b=h0 of the following group -> the
                # scan state re-initializes exactly at each group boundary
                for i in range(n - 1):
                    rc = i * STRIDE + T
                    nc.scalar.copy(at[:, rc : rc + 1], zcol[:, :])
                    nc.scalar.copy(
                        bt[:, rc : rc + 1], h0t[:, mg[i + 1] : mg[i + 1] + 1]
                    )
                if mg[-1] == NG - 1:
                    # tail: split so the first half's output DMA overlaps
                    for i, (c0, c1) in enumerate([(0, HALF), (HALF, T)]):
                        init = (
                            h0t[:, g0 : g0 + 1] if i == 0
                            else ht[:, c0 - 1 : c0]
                        )
                        nc.vector.tensor_tensor_scan(
                            out=ht[:, c0:c1], data0=at[:, c0:c1],
                            data1=bt[:, c0:c1],
                            initial=init, op0=MULT, op1=ADD,
                        )
                        rows = slice((NG - 1) * G, NG * G)
                        nc.scalar.dma_start(
                            out=y_d[rows, c0:c1], in_=ht[:, c0:c1]
                        )
                    continue
                nc.vector.tensor_tensor_scan(
                    out=ht[:, :], data0=at[:, :], data1=bt[:, :],
                    initial=h0t[:, g0 : g0 + 1], op0=MULT, op1=ADD,
                )
                for i, g in enumerate(mg):
                    rows = slice(g * G, (g + 1) * G)
                    cl = slice(i * STRIDE, i * STRIDE + T)
                    nc.scalar.dma_start(out=y_d[rows, :], in_=ht[:, cl])
    if not nc.is_finalized():
        nc.finalize()
    return nc


def run(inputs: dict, trace: bool = False, tmpdir=None) -> tuple[np.ndarray, object]:
    f = np.asarray(inputs["f"], dtype=np.float32)
    x = np.asarray(inputs["x"], dtype=np.float32)
    h0 = np.asarray(inputs["hidden_init"], dtype=np.float32)

    # Host-side prep is layout/dtype only: [T, B, H] fp32 -> per-core
    # channel-major [C, T] fp16.
    ftr = f.astype(np.float16).transpose(1, 2, 0)  # (B, H, T)
    xtr = x.astype(np.float16).transpose(1, 2, 0)

    nc = build_program()
    in_maps = []
    for m in range(NCORES):
        sl = slice(m * BS, (m + 1) * BS)
        in_maps.append(
            {
                "f": np.ascontiguousarray(ftr[sl]).reshape(C, T),
                "x": np.ascontiguousarray(xtr[sl]).reshape(C, T),
                "h0": np.ascontiguousarray(h0[sl].reshape(NG, G).T),
            }
        )
    res = bass_utils.run_bass_kernel_spmd(
        nc, in_maps, core_ids=list(range(NCORES)), trace=trace, tmpdir=tmpdir
    )
    # y arrives [C, T] fp16 per core; restore [T, BS, H] fp32
    outs = [r["y"].reshape(BS, H, T).transpose(2, 0, 1) for r in res.results]
    return np.concatenate(outs, axis=1).astype(np.float32), res


def kernel(**inputs) -> np.ndarray:
    out, _ = run(inputs, trace=False)
    return out


# revision 22
# speedup vs baseline: 1.0245x; 1.0245x over previous
"""ForgetMult linear recurrence h_t = f_t*x_t + (1-f_t)*h_{t-1} on 8 trn2 cores.

Sharding: batch dim B=64 split across 8 cores (8 batches/core). Per core the
C = 8*1024 = 8192 (b,h) channels are independent scans over T=1024.

All wire traffic fp16 (host pre-transposes to channel-major [C, T]).
Everything runs on DVE + ACT only: concurrent GpSimd SBUF traffic slows DVE
scans ~2.3x (SBUF port contention), and at the quiet rate the scan costs
~187ns + 2.02 cyc/elem.

Groups are processed in merged scans: several groups share one long scan
instruction, separated by reset columns with a=0, b=h0_next — the scan
state = a*state + b makes that an EXACT re-initialization. This amortizes
per-instruction overhead for both the scan and the 2x-mode b=f*x multiply.
"""

import numpy as np

import concourse.bacc as bacc
import concourse.bass as bass
import concourse.mybir as mybir
from concourse import bass_utils
from concourse.tile import TileContext

T = 1024
B = 64
H = 1024
NCORES = 8
BS = B // NCORES  # batches per core
C = BS * H  # channels per core (independent scans)
G = 128  # channels per group == partition dim
NG = C // G  # channel groups per core
STRIDE = T + 1  # group data + one reset column

F32 = mybir.dt.float32
F16 = mybir.dt.float16

# Merge plan: singles first so the pipeline ramps quickly (group 0 is fed
# in T-halves), 4-group merges in steady state, singles at the end (the
# last one split so its output DMA overlaps its final chunk's scan).
MERGES = [[0], [1]] + [list(range(2 + 4 * k, 6 + 4 * k)) for k in range(15)]
MERGES += [[62], [63]]


def build_program() -> bass.Bass:
    nc = bacc.Bacc(trn_type="TRN2")
    f_d = nc.dram_tensor("f", (C, T), F16, kind="ExternalInput")
    x_d = nc.dram_tensor("x", (C, T), F16, kind="ExternalInput")
    h0_d = nc.dram_tensor("h0", (G, NG), F32, kind="ExternalInput")
    y_d = nc.dram_tensor("y", (C, T), F16, kind="ExternalOutput")

    COPY = mybir.ActivationFunctionType.Copy
    MULT = mybir.AluOpType.mult
    ADD = mybir.AluOpType.add
    HALF = T // 2

    with TileContext(nc) as tc:
        with (
            tc.tile_pool(name="consts", bufs=1) as consts,
            tc.tile_pool(name="io", bufs=5) as io,
            tc.tile_pool(name="mid", bufs=4) as mid,
            tc.tile_pool(name="hpool", bufs=3) as hpool,
        ):
            h0t = consts.tile([G, NG], F32)
            zcol = consts.tile([G, 1], F16)
            nc.gpsimd.memset(zcol[:, :], 0.0)

            for mg in MERGES:
                n = len(mg)
                W = n * STRIDE - 1  # last group needs no reset column
                g0 = mg[0]
                ft = io.tile([G, W], F16, tag="f", name=f"f{g0}")
                xt = io.tile([G, W], F16, tag="x", name=f"x{g0}")
                at = mid.tile([G, W], F16, tag="a", name=f"a{g0}")
                bt = mid.tile([G, W], F16, tag="b", name=f"b{g0}")
                ht = hpool.tile([G, W], F16, tag="h", name=f"h{g0}")
                if g0 == 0:
                    # Ramp: the first scan gates the DVE-bound kernel; feed
                    # group 0 in T-halves with everything on DVE/ACT.
                    rows = slice(0, G)
                    for i in range(2):
                        tl = slice(i * HALF, (i + 1) * HALF)
                        nc.sync.dma_start(out=ft[:, tl], in_=f_d[rows, tl])
                        nc.sync.dma_start(out=xt[:, tl], in_=x_d[rows, tl])
                        if i == 0:
                            # h0 load rides behind the first data tiles
                            nc.sync.dma_start(out=h0t[:, :], in_=h0_d[:, :])
                        nc.scalar.activation(
                            at[:, tl], ft[:, tl], COPY, bias=1.0, scale=-1.0
                        )
                        nc.vector.tensor_tensor(
                            out=bt[:, tl], in0=ft[:, tl], in1=xt[:, tl],
                            op=MULT,
                        )
                        init = (
                            h0t[:, 0:1] if i == 0 else ht[:, HALF - 1 : HALF]
                        )
                        nc.vector.tensor_tensor_scan(
                            out=ht[:, tl], data0=at[:, tl], data1=bt[:, tl],
                            initial=init, op0=MULT, op1=ADD,
                        )
                    nc.scalar.dma_start(out=y_d[rows, :], in_=ht[:, :])
                    continue
                for i, g in enumerate(mg):
                    rows = slice(g * G, (g + 1) * G)
                    cl = slice(i * STRIDE, i * STRIDE + T)
                    nc.sync.dma_start(out=ft[:, cl], in_=f_d[rows, :])
                    nc.sync.dma_start(out=xt[:, cl], in_=x_d[rows, :])
                    # a = 1 - f on the data slice only (reset cols untouched)
                    nc.scalar.activation(
                        at[:, cl], ft[:, cl], COPY, bias=1.0, scale=-1.0
                    )
                # b = f*x full width on DVE 2x mode (junk in reset cols)
                nc.vector.tensor_tensor(
                    out=bt[:, :], in0=ft[:, :], in1=xt[:, :], op=MULT
                )
                # reset columns: a=0, b=h0 of the following group -> the
                # scan state re-initializes exactly at each group boundary
                for i in range(n - 1):
                    rc = i * STRIDE + T
                    # on the DVE queue: no cross-engine sem hop ahead of the
                    # scan that consumes them
                    nc.vector.tensor_tensor(
                        out=at[:, rc : rc + 1], in0=zcol[:, :], in1=zcol[:, :],
                        op=mybir.AluOpType.bypass,
                    )
                    nc.vector.tensor_tensor(
                        out=bt[:, rc : rc + 1],
                        in0=h0t[:, mg[i + 1] : mg[i + 1] + 1],
                        in1=zcol[:, :],
                        op=mybir.AluOpType.bypass,
                    )
                if mg[-1] == NG - 1:
                    # tail: split so most of the output DMA overlaps the
                    # final short chunk's scan
                    for i, (c0, c1) in enumerate([(0, 768), (768, T)]):
                        init = (
                            h0t[:, g0 : g0 + 1] if i == 0
                            else ht[:, c0 - 1 : c0]
                        )
                        nc.vector.tensor_tensor_scan(
                            out=ht[:, c0:c1], data0=at[:, c0:c1],
                            data1=bt[:, c0:c1],
                            initial=init, op0=MULT, op1=ADD,
                        )
                        rows = slice((NG - 1) * G, NG * G)
                        nc.scalar.dma_start(
                            out=y_d[rows, c0:c1], in_=ht[:, c0:c1]
                        )
                    continue
                nc.vector.tensor_tensor_scan(
                    out=ht[:, :], data0=at[:, :], data1=bt[:, :],
                    initial=h0t[:, g0 : g0 + 1], op0=MULT, op1=ADD,
                )
                for i, g in enumerate(mg):
                    rows = slice(g * G, (g + 1) * G)
                    cl = slice(i * STRIDE, i * STRIDE + T)
                    nc.scalar.dma_start(out=y_d[rows, :], in_=ht[:, cl])
    if not nc.is_finalized():
        nc.finalize()
    return nc


def run(inputs: dict, trace: bool = False, tmpdir=None) -> tuple[np.ndarray, object]:
    f = np.asarray(inputs["f"], dtype=np.float32)
    x = np.asarray(inputs["x"], dtype=np.float32)
    h0 = np.asarray(inputs["hidden_init"], dtype=np.float32)

    # Host-side prep is layout/dtype only: [T, B, H] fp32 -> per-core
    # channel-major [C, T] fp16.
    ftr = f.astype(np.float16).transpose(1, 2, 0)  # (B, H, T)
    xtr = x.astype(np.float16).transpose(1, 2, 0)

    nc = build_program()
    in_maps = []
    for m in range(NCORES):
        sl = slice(m * BS, (m + 1) * BS)
        in_maps.append(
            {
                "f": np.ascontiguousarray(ftr[sl]).reshape(C, T),
                "x": np.ascontiguousarray(xtr[sl]).reshape(C, T),
                "h0": np.ascontiguousarray(h0[sl].reshape(NG, G).T),
            }
        )
    res = bass_utils.run_bass_kernel_spmd(
        nc, in_maps, core_ids=list(range(NCORES)), trace=trace, tmpdir=tmpdir
    )
    # y arrives [C, T] fp16 per core; restore [T, BS, H] fp32
    outs = [r["y"].reshape(BS, H, T).transpose(2, 0, 1) for r in res.results]
    return np.concatenate(outs, axis=1).astype(np.float32), res


def kernel(**inputs) -> np.ndarray:
    out, _ = run(inputs, trace=False)
    return out
